# revision 44
# baseline (speedup 1.0000x reference)
"""Trainium2 Bass kernel for nn_Decoder (NeRF-style 9-layer MLP, Softplus(beta=100)).

Strategy (pure data parallel over 8 cores, feature-major layout):
  - activations live in SBUF as z_l = 100 * y_l (scaled softplus outputs), shape
    [features<=100 partitions, points free-dim]; weights are stationary lhsT.
  - matmuls run in float32r (1 cycle/row for free-dim >= 256, 4x faster than
    plain fp32; numerically exact in fp32 accumulation).
  - per layer, per supertile [100, T]:
      PE : psum = W z                        (T/512 matmuls, N=512 each)
      ACT: s = Sigmoid(-psum - 100b)         (= sigma(-u), one table op)
      DVE: z' = max(u + g, g)                (one fused custom op)
            where u = psum + 100b,  p = min(s, 1-s),
                  g = (c2*p + c1)*p  ~=  -ln(1-p)  (minimax on [0, 1/2])
    Math: softplus(u) = relu(u) + (-ln(1 - min(sigma(-u), sigma(u)))).
    The deg-2 minimax of -ln(1-p) has max err 5.4e-3 in z units (5.4e-5 in y),
    end-to-end rel err ~7e-4. Exact in both tails (p->0 as |u|->inf).
  - skip connection (layer 4) handled by DMAing the raw input into partitions
    98:100 of the layer-3 output tile; layer-4 weight columns scaled to match.
  - layer 8 (100->1, no activation): matmul into psum row 0, DMA straight to
    DRAM; the scalar bias b8 is added on the host after the gather.
Supertiles are emitted software-pipelined in groups of GRP=4 (layers
interleaved across the group) so each engine's in-order stream never
head-of-line blocks on the serial MM->ACT->DVE chain of a single supertile.
"""

import numpy as np

import concourse.bass as bass
import concourse.tile as tile
from concourse import bacc, mybir
from concourse import bass_utils
from concourse.bass_interp import get_hw_module

F32 = mybir.dt.float32
F32R = mybir.dt.float32r
ACTF = mybir.ActivationFunctionType

N_CORES = 8
N_TOTAL = 1048576
P = N_TOTAL // N_CORES          # 131072 points per core
DIMS = [2, 100, 100, 100, 98, 100, 100, 100, 100, 1]

# deg-2 minimax of -ln(1-p) on [0, 0.5]: g = (SP_C2*p + SP_C1)*p
SP_C1 = 0.94427875
SP_C2 = 0.86235463

_SOFTPLUS_SIG = None


def _wpack_layout():
    """Column layout of the packed weight/bias tensor [100, WPK].  Layer 0's
    weights and biases occupy the first columns so a slim head DMA can launch
    the pipeline while the rest streams in."""
    wcol = [0] * 8
    bncol = [0] * 8
    bpcol = [0] * 8
    c = DIMS[1]          # lhsT0
    bncol[0] = c
    bpcol[0] = c + 1
    c += 2
    head = c             # end of the slim head
    for l in range(1, 8):
        wcol[l] = c
        c += DIMS[l + 1]
    w8col = []
    for i in range(16):
        w8col.append(c)
        c += 16
    for l in range(1, 8):
        bncol[l] = c
        bpcol[l] = c + 1
        c += 2
    return wcol, w8col, bncol, bpcol, head, c


def _get_softplus_sig():
    """Register (once) the fused custom-DVE op computing, per element,
        z = max(u + g, g)   with u = in1 + s0,  p = min(in0, 1-in0),
                                 g = (imm2*p + s1)*p
    in0 = sigma(-u) tile (SBUF), in1 = psum (PSUM), s0 = +100*b [P,1],
    s1 = SP_C1, imm2 = SP_C2.  Exactly 8 ALU stages (v3 pipeline depth)."""
    global _SOFTPLUS_SIG
    if _SOFTPLUS_SIG is not None:
        return _SOFTPLUS_SIG
    from concourse import dve_ops
    from concourse.dve_spec import (
        Spec, Src0, Src1, C0, C1, C2, One, lower, maxx, minn, _has_src1,
    )
    from concourse.dve_uop import DveOpSpec

    name = "SOFTPLUS_SIG_ANT"
    _p = minn(Src0, One - Src0)
    _u = Src1 + C0
    _g = (C2 * _p + C1) * _p
    body = maxx(_u + _g, _g)

    def _ref(in0, in1, s0, s1, imm2):
        s = np.asarray(in0, dtype=np.float32)
        u = (np.asarray(in1, dtype=np.float32) + np.asarray(s0, np.float32)).astype(
            np.float32
        )
        w = (np.float32(1.0) - s).astype(np.float32)
        p = np.minimum(s, w)
        g = (
            (np.float32(imm2) * p + np.asarray(s1, np.float32)).astype(np.float32) * p
        ).astype(np.float32)
        return np.maximum((u + g).astype(np.float32), g)

    spec = Spec(body=body, reference=_ref)
    op = dve_ops.DveOp(name, spec, subdim=False, uops_sha={})
    dve_ops.OPS.append(op)
    dve_ops.CUSTOM_DVE_SPECS[name] = spec
    dve_ops._SUB_OPCODE_FOR_NAME[name] = (
        dve_ops._CUSTOM_DVE_ROW_BASE + len(dve_ops.OPS) - 1
    )
    assert dve_ops._SUB_OPCODE_FOR_NAME[name] < 0x20
    for ver in ("v3", "v4"):
        uops = lower(spec, ver=ver)
        tmp = DveOpSpec(
            name=name,
            opcode=dve_ops.get_dve_sub_opcode(name),
            uops=uops,
            rd1_en=_has_src1(spec),
        )
        op.uops_sha[ver] = tmp.sha(ver)
    _SOFTPLUS_SIG = op
    return _SOFTPLUS_SIG


def _build_program(T=1024, psum_bufs=4, sbufs=8, mbufs=16, GRP=4, xbufs=6):
    NT = P // T
    sp_sig = _get_softplus_sig()
    nc = bacc.Bacc(
        "TRN2",
        target_bir_lowering=False,
        debug=False,
        enable_asserts=False,
        num_devices=N_CORES,
    )

    # DRAM I/O (per core).  All weights/biases arrive in ONE packed [100, WPK]
    # tensor (one DMA instead of ~30 -- the serial HWDGE preamble was 20us):
    # columns [wcol[l] : wcol[l]+out_dim] hold lhsT_l (valid on partitions
    # 0:in_dim), then 4 one-hot layer-8 weight blocks of 4 columns each, then
    # 8 columns of bneg and 8 of bpos (valid on partitions 0:out_dim).
    wcol, w8col, bncol, bpcol, WHEAD, WPK = _wpack_layout()
    xt_d = nc.dram_tensor("xt", [2, P], F32, kind="ExternalInput")
    wpk_d = nc.dram_tensor("wpack", [100, WPK], F32, kind="ExternalInput")
    y_d = nc.dram_tensor("y", [1, P], F32, kind="ExternalOutput")

    with tile.TileContext(nc) as tc:
        with (
            tc.tile_pool(name="wpool", bufs=1) as wpool,
            tc.tile_pool(name="xpool", bufs=xbufs) as xpool,
            tc.tile_pool(name="psum", bufs=psum_bufs, space="PSUM") as pspool,
            tc.tile_pool(name="spool", bufs=sbufs) as spool,
            tc.tile_pool(name="mpool", bufs=mbufs) as mpool,
            tc.tile_pool(name="opool", bufs=4) as opool,
        ):
            # dummy 1-element sigmoid with no upstream deps: walrus places
            # the ACT table load before it, so the load runs at t~0 instead
            # of gating the first real sigmoid
            dz = wpool.tile([1, 1], F32, tag="dz")
            nc.vector.memset(dz[:], 0.0)
            dzo = wpool.tile([1, 1], F32, tag="dzo")
            nc.scalar.activation(dzo[:], dz[:], ACTF.Sigmoid, bias=0.0, scale=1.0)
            # dummy matmuls warm the PE p-state while the input/weight DMAs
            # are in flight, so the first real matmuls run at mid/full clock
            dmm = wpool.tile([1, 512], F32R, tag="dmm")
            nc.vector.memset(dmm[:].bitcast(F32), 0.0)
            dps = pspool.tile([100, T], F32, tag="ps")
            for _ in range(WARM):
                nc.tensor.matmul(
                    dps[0:1, 0:512], dmm[0:1, 0:1], dmm[0:1, 0:512],
                    start=True, stop=True,
                )
            # --- first input tile, then weights: slim head (layer-0 weights
            # and biases) first so the first matmul chain launches early
            xt0 = xpool.tile([2, T], F32R, tag="xt")
            nc.sync.dma_start(xt0[:], xt_d.ap()[:, bass.ts(0, T)].bitcast(F32R))
            wpk = wpool.tile([100, WPK], F32R, tag="wpack")
            # slim head on the ACT HWDGE queue: overlaps xt0 on the SP queue
            # (the table load was already hoisted ahead of it by the dummy)
            nc.scalar.dma_start(wpk[0:100, 0:WHEAD], wpk_d.ap()[:, 0:WHEAD].bitcast(F32R))
            nc.sync.dma_start(wpk[0:100, WHEAD:WPK], wpk_d.ap()[:, WHEAD:WPK].bitcast(F32R))
            wts = []
            for l in range(8):
                in_dim = 100 if l == 4 else DIMS[l]
                out_dim = DIMS[l + 1]
                wts.append(wpk[0:in_dim, wcol[l] : wcol[l] + out_dim])
            w8ts = [wpk[0:100, w8col[i] : w8col[i] + 16] for i in range(16)]
            bnegs = [
                wpk[0 : DIMS[l + 1], bncol[l] : bncol[l] + 1].bitcast(F32)
                for l in range(8)
            ]
            bposs = [
                wpk[0 : DIMS[l + 1], bpcol[l] : bpcol[l] + 1].bitcast(F32)
                for l in range(8)
            ]

            # --- main loop ---
            assert NT % GRP == 0 and GRP == 4

            def emit_l8(col0, prevs, final=False):
                # accumulate the GRP supertiles' scalar outputs into a single
                # [16, 256] psum tile (row 4i+j = supertile i, quarter j) via
                # one-hot weight columns; the ACT eviction copy then has free
                # size 256 so it barely perturbs the sigma stream.
                nq = T // 256
                ps = pspool.tile([100, T], F32, tag="ps")
                k = 0
                for i in range(GRP):
                    for j in range(nq):
                        js = bass.ts(j, 256)
                        nc.tensor.matmul(
                            ps[0:16, 0:256], w8ts[k], prevs[i][0:100, js],
                            start=(k == 0), stop=(k == GRP * nq - 1),
                        )
                        k += 1
                out_t = opool.tile([16, 256], F32, tag="out")
                nc.scalar.copy(out_t[:], ps[0:16, 0:256])
                # one DMA for the whole group: row 4i+j of out_t lands at
                # y[col0 + 1024*i + 256*j : +256] -- rows map contiguously
                nc.sync.dma_start(
                    y_d.ap()[:, col0 : col0 + GRP * T], out_t[0:16, :]
                )

            deferred = None
            for g in range(NT // GRP):
                ts_ids = [g * GRP + i for i in range(GRP)]
                sls = [bass.ts(t, T) for t in ts_ids]
                xts = []
                for i, sl in enumerate(sls):
                    if g == 0 and i == 0:
                        xts.append(xt0)
                        continue
                    xt = xpool.tile([2, T], F32R, tag="xt")
                    nc.sync.dma_start(xt[:], xt_d.ap()[:, sl].bitcast(F32R))
                    xts.append(xt)
                prevs = list(xts)
                for l in range(8):
                    if l == DEFER and deferred is not None:
                        emit_l8(*deferred)
                        deferred = None
                    in_dim = 100 if l == 4 else DIMS[l]
                    out_dim = DIMS[l + 1]
                    pss = []
                    for i in range(GRP):
                        ps = pspool.tile([100, T], F32, tag="ps")
                        for j in range(T // 512):
                            js = bass.ts(j, 512)
                            rhs = prevs[i][0:in_dim, js]
                            nc.tensor.matmul(
                                ps[0:out_dim, js],
                                wts[l],
                                rhs,
                                start=True,
                                stop=True,
                            )
                        pss.append(ps)
                    sts = []
                    for i in range(GRP):
                        st = spool.tile([100, T], F32, tag="s")
                        nc.scalar.activation(
                            st[0:out_dim, :], pss[i][0:out_dim, :], ACTF.Sigmoid,
                            bias=bnegs[l], scale=-1.0,
                        )
                        sts.append(st)
                    nprevs = []
                    for i in range(GRP):
                        m = mpool.tile([100, T], F32R, tag="m7" if l == 7 else "m")
                        out_ap = m[0:out_dim, :]
                        if l == 3:
                            nc.sync.dma_start(
                                m[98:100, :], xt_d.ap()[:, sls[i]].bitcast(F32R)
                            )
                        nc.vector._custom_dve(
                            sp_sig,
                            out=out_ap,
                            in0=sts[i][0:out_dim, :],
                            in1=pss[i][0:out_dim, :],
                            s0=bposs[l],
                            s1=SP_C1,
                            imm2=SP_C2,
                        )
                        nprevs.append(m)
                    prevs = nprevs
                if deferred is not None:
                    emit_l8(*deferred)
                deferred = (ts_ids[0] * T, prevs)
            if deferred is not None:
                emit_l8(*deferred, final=True)

    nc.compile()
    nc.m = get_hw_module(nc.m)
    return nc


def _transform_weights(inputs):
    """Host-side weight/bias transform -> one packed DRAM tensor (shared
    across cores)."""
    W = [np.asarray(inputs[f"W{l}"], dtype=np.float32) for l in range(9)]
    b = [np.asarray(inputs[f"b{l}"], dtype=np.float32) for l in range(9)]
    wcol, w8col, bncol, bpcol, WHEAD, WPK = _wpack_layout()
    pk = np.zeros((100, WPK), dtype=np.float32)
    lhsT = {}
    lhsT[0] = (100.0 * W[0]).T.astype(np.float32)
    for l in (1, 2, 3, 5, 6, 7):
        lhsT[l] = W[l].T
    # layer-4 input tile layout: partitions 0:98 = z3, 98:100 = raw x
    lhsT[4] = np.concatenate(
        [W[4][:, 2:].T, (100.0 * W[4][:, :2]).T.astype(np.float32)], axis=0
    )
    for l in range(8):
        t = lhsT[l]
        pk[0 : t.shape[0], wcol[l] : wcol[l] + t.shape[1]] = t
    for i in range(16):
        pk[:, w8col[i] + i] = W[8].reshape(-1) / 100.0
    for l in range(8):
        bn = (-100.0 * b[l]).astype(np.float32)
        bp = (100.0 * b[l]).astype(np.float32)
        pk[0 : bn.size, bncol[l]] = bn
        pk[0 : bp.size, bpcol[l]] = bp
    return {"wpack": pk}


_NC_CACHE = None


def kernel(**inputs) -> np.ndarray:
    global _NC_CACHE
    if _NC_CACHE is None:
        _NC_CACHE = _build_program()
    nc = _NC_CACHE

    x = np.asarray(inputs["input"], dtype=np.float32)
    assert x.shape == (N_TOTAL, 2)
    shared = _transform_weights(inputs)

    in_maps = []
    for c in range(N_CORES):
        m = dict(shared)
        m["xt"] = np.ascontiguousarray(x[c * P : (c + 1) * P].T)
        in_maps.append(m)

    res = bass_utils.run_bass_kernel_spmd(nc, in_maps, core_ids=list(range(N_CORES)))
    b8 = np.float32(np.asarray(inputs["b8"], dtype=np.float32).reshape(()))
    y = np.concatenate([res.results[c]["y"][0] for c in range(N_CORES)])
    y = (y + b8).astype(np.float32)
    return y.reshape(N_TOTAL, 1)
